# revision 17
# baseline (speedup 1.0000x reference)
"""Multi-head attention (B=4, L=2048, D=1024, H=16) on 8 TRN2 NeuronCores.

Sharding: core c handles batch b=c//2, query half qh=c%2 (1024 query tokens,
all heads, full 2048-key context). K/V projections are duplicated across the
2 cores sharing a batch; no cross-core communication needed.

Per-core dataflow:
  - Q.T/K.T projections drained to fp8 (e4m3) with bias, then DMA-remapped
    into per-m slabs [32|32, 2, tokens] so the dk=64 contraction runs as a
    DoubleRow fp8 matmul (0.5 cycles/row) on a 32-partition strip.
  - scores S.T[k,q] per (head, key-block j): 2 DoubleRow matmuls -> PSUM
  - E = exp(S/8) on ScalarE (the only engine with exp; it is the wall)
  - E *= mask.T on VectorE/GpSimd (split for balance)
  - AV flipped: per (h, q-tile t): out[q, 65] = sum_j E_j-block.T @ V-slot
    (V slots are 65 wide: 64 dk cols + ones col -> col 64 = softmax denom)
  - normalize: C = AV[:,0:64] / AV[:,64] via GpSimd tensor_scalar divide
  - C.T via DMA-transpose (XBAR) into CT tiles
  - out[q,1024] = CT-chunks.T @ Wo.T + bo_eff, bias via ones-row matmul

Schedule: per-head windows paced by ScalarE exp; projections (m-ordered
units) trickle into the PE stream as fillers with per-window deadlines;
AV(h-1) runs as split bursts early in window h.
"""

import sys
import functools
from collections import deque

sys.path.insert(0, "/opt/trn_rl_repo")

import numpy as np
import ml_dtypes

BF16NP = ml_dtypes.bfloat16
F8NP = ml_dtypes.float8_e4m3

B, L, D, H, DK = 4, 2048, 1024, 16, 64
NCORES = 8
LQ = L // 2          # query tokens per core
NI = D // 128        # input-dim chunks
NM = D // 128        # dk-dim m-tiles (2 heads each)
NJ = L // 128        # key tiles
SLOT = DK + 1        # V slot width (64 cols + ones)
VW = H * SLOT        # 1040


def _build():
    import concourse.mybir as mybir
    import concourse.tile as tile
    from concourse import bacc

    dt = mybir.dt
    F32, BF, F8 = dt.float32, dt.bfloat16, dt.float8e4
    AF = mybir.ActivationFunctionType
    DR = mybir.MatmulPerfMode.DoubleRow
    DIVOP = mybir.AluOpType.divide

    nc = bacc.Bacc("TRN2", target_bir_lowering=False, debug=False,
                   num_devices=NCORES, dynamic_dma_scratch_size=1024)

    xq_d = nc.dram_tensor("xq", [NI, 128, LQ], BF, kind="ExternalInput")
    xk_d = nc.dram_tensor("xk", [NI, 128, L], BF, kind="ExternalInput")
    xv_d = nc.dram_tensor("xv", [NI, 128, L], BF, kind="ExternalInput")
    wq_d = nc.dram_tensor("wq", [NI, 128, D], BF, kind="ExternalInput")
    wk_d = nc.dram_tensor("wk", [NI, 128, D], BF, kind="ExternalInput")
    wv_d = nc.dram_tensor("wv", [NI, 128, D], BF, kind="ExternalInput")
    wo_d = nc.dram_tensor("wo", [NI, 128, D], BF, kind="ExternalInput")
    mt_d = nc.dram_tensor("maskt", [NJ, 128, LQ], BF, kind="ExternalInput")
    bq_d = nc.dram_tensor("bqt", [128, NM], F32, kind="ExternalInput")
    bk_d = nc.dram_tensor("bkt", [128, NM], F32, kind="ExternalInput")
    bo_d = nc.dram_tensor("bor", [1, D], BF, kind="ExternalInput")
    out_d = nc.dram_tensor("out", [NM, 128, D], BF, kind="ExternalOutput")

    keep = []

    def single(shape, dtyp, name):
        t, free = tc.tile(shape, dtyp, name=name)
        keep.append(free)
        return t

    with tile.TileContext(nc) as tc:
        # ---- persistent tiles ----
        VP = [single([128, VW], BF, f"vp{j}") for j in range(NJ)]
        CT = [single([128, LQ], BF, f"ct{m}") for m in range(NM)]
        MTA = single([128, NJ * LQ], BF, "mta")
        XQA = single([128, NI * LQ], BF, "xqa")
        XKA = single([128, NI * L], BF, "xka")
        warm_sb = single([128, 512], BF, "warm_sb")
        bq_sb = single([128, NM], F32, "bq_sb")
        bk_sb = single([128, NM], F32, "bk_sb")
        bo_sb = single([1, D], BF, "bo_sb")
        ones_row = single([1, 128], BF, "ones_row")

        nc.vector.memset(ones_row[:], 1.0)
        nc.vector.memset(warm_sb[:], 0.0)

        with (
            tc.tile_pool(name="q8p", bufs=2) as q8p,
            tc.tile_pool(name="k8p", bufs=2) as k8p,
            tc.tile_pool(name="f8p", bufs=2) as f8p,
            tc.tile_pool(name="ep", bufs=20) as ep,
            tc.tile_pool(name="wlp", bufs=3) as wlp,
            tc.tile_pool(name="wvp", bufs=2) as wvp,
            tc.tile_pool(name="xvp", bufs=2) as xvp,
            tc.tile_pool(name="cnp", bufs=2) as cnp,
            tc.tile_pool(name="sp", bufs=2, space="PSUM") as sp,
            tc.tile_pool(name="avp", bufs=2, space="PSUM") as avp,
            tc.tile_pool(name="wkp", bufs=2, space="PSUM") as wkp,
        ):
            nc.sync.dma_start(bq_sb[:], bq_d.ap())
            nc.sync.dma_start(bk_sb[:], bk_d.ap())
            nc.sync.dma_start(bo_sb[:], bo_d.ap())

            Q8 = {}
            K8 = {}

            # ---------- projection unit emitters ----------
            def q_proj(m, xsrc=None):
                wt = wlp.tile([128, NI * 128], BF, tag="w", name=f"wq{m}")
                nc.sync.dma_start(
                    wt[:].rearrange("p (i c) -> p i c", c=128),
                    wq_d.ap()[:, :, m * 128:(m + 1) * 128].rearrange(
                        "i p c -> p i c"))
                qf = f8p.tile([128, L], F8, tag="f", name=f"qf{m}")
                for c in range(2):
                    if xsrc is None:
                        xs = lambda i: XQA[:, i * LQ + c * 512:i * LQ + (c + 1) * 512]
                    else:
                        xs = xsrc(c)
                    ps = wkp.tile([128, 512], F32, tag="k", name=f"psq{m}{c}")
                    for i in range(NI):
                        nc.tensor.matmul(
                            ps[:], wt[:, i * 128:(i + 1) * 128], xs(i),
                            start=(i == 0), stop=(i == NI - 1))
                    cs = slice(c * 512, (c + 1) * 512)
                    nc.vector.tensor_scalar_add(qf[:, cs], ps[:], bq_sb[:, m:m + 1])
                q8 = q8p.tile([64, 2 * LQ], F8, tag="q8", name=f"q8_{m}")
                Q8[m] = q8
                for p2 in range(2):
                    for i2 in range(2):
                        src0 = 64 * p2 + 32 * i2
                        nc.scalar.dma_start(
                            q8[32 * p2:32 * p2 + 32, i2 * LQ:(i2 + 1) * LQ],
                            qf[src0:src0 + 32, 0:LQ])

            def k_proj_units(m, xsrc=None):
                # 4 units of (m, c); last one does the remap
                wts = []
                kf_box = []

                def unit(c):
                    if c == 0:
                        wt = wlp.tile([128, NI * 128], BF, tag="w", name=f"wk{m}")
                        nc.sync.dma_start(
                            wt[:].rearrange("p (i c) -> p i c", c=128),
                            wk_d.ap()[:, :, m * 128:(m + 1) * 128].rearrange(
                                "i p c -> p i c"))
                        wts.append(wt)
                        kf_box.append(f8p.tile([128, L], F8, tag="f", name=f"kf{m}"))
                    wt = wts[0]
                    kf = kf_box[0]
                    if xsrc is None:
                        xs = lambda i: XKA[:, i * L + c * 512:i * L + (c + 1) * 512]
                    else:
                        xs = xsrc(c)
                    cs = slice(c * 512, (c + 1) * 512)
                    ps = wkp.tile([128, 512], F32, tag="k", name=f"psk{m}{c}")
                    for i in range(NI):
                        nc.tensor.matmul(
                            ps[:], wt[:, i * 128:(i + 1) * 128], xs(i),
                            start=(i == 0), stop=(i == NI - 1))
                    nc.vector.tensor_scalar_add(kf[:, cs], ps[:], bk_sb[:, m:m + 1])
                    if c == 3:
                        k8 = k8p.tile([64, 2 * L], F8, tag="k8", name=f"k8_{m}")
                        K8[m] = k8
                        for p2 in range(2):
                            for i2 in range(2):
                                nc.scalar.dma_start(
                                    k8[32 * p2:32 * p2 + 32, i2 * L:(i2 + 1) * L],
                                    kf[64 * p2 + 32 * i2:64 * p2 + 32 * i2 + 32, :])
                return [(f"K{m}c{c}", 4096, functools.partial(unit, c))
                        for c in range(4)]

            vdone = set()

            def v_pass_units(a):
                # dk cols [256a, 256a+256) = heads 4a..4a+3, j-inner
                wvs = []
                xs_state = {}

                def load_ws():
                    wt = wvp.tile([128, NI * 256], BF, tag="wv", name=f"wv{a}")
                    nc.sync.dma_start(
                        wt[:].rearrange("p (i c) -> p i c", c=256),
                        wv_d.ap()[:, :, 256 * a:256 * a + 256].rearrange(
                            "i p c -> p i c"))
                    wvs.append(wt)

                def load_xs(c):
                    xt = xvp.tile([128, NI * 512], BF, tag="xv",
                                  name=f"xv{a}_{c}")
                    nc.sync.dma_start(
                        xt[:].rearrange("p (i t) -> p i t", t=512),
                        xv_d.ap()[:, :, c * 512:(c + 1) * 512].rearrange(
                            "i p t -> p i t"))
                    xs_state[c] = xt

                def unit(c, jj):
                    if c == 0 and jj == 0:
                        load_ws()
                    if jj == 0:
                        load_xs(c)
                    xt = xs_state[c]
                    wt = wvs[0]
                    j = c * 4 + jj
                    ps = wkp.tile([128, 512], F32, tag="k", name=f"psv{a}_{j}")
                    for i in range(NI):
                        nc.tensor.matmul(
                            ps[:, 0:256],
                            xt[:, i * 512 + jj * 128:i * 512 + (jj + 1) * 128],
                            wt[:, i * 256:(i + 1) * 256],
                            start=(i == 0), stop=(i == NI - 1))
                    dst = VP[j][:].rearrange("p (h w) -> p h w", w=SLOT)[
                        :, 4 * a:4 * a + 4, 0:DK]
                    src = ps[:, 0:256].rearrange("p (h w) -> p h w", w=DK)
                    nc.vector.tensor_copy(dst, src)
                    if a == 0:
                        nc.vector.memset(VP[j][:, DK::SLOT], 1.0)
                    if c == 3 and jj == 3:
                        vdone.add(a)

                return [(f"V{a}j{c * 4 + jj}", 2048,
                         functools.partial(unit, c, jj))
                        for c in range(4) for jj in range(4)]

            # ---------- filler queue with deadline ordering ----------
            fillers = deque()
            fillers.extend(v_pass_units(0))                       # heads 0-3
            fillers.append(("Q1", 8192, functools.partial(q_proj, 1)))
            fillers.extend(k_proj_units(1))
            fillers.append(("Q2", 8192, functools.partial(q_proj, 2)))
            fillers.extend(k_proj_units(2))
            fillers.extend(v_pass_units(1))                       # heads 4-7
            fillers.append(("Q3", 8192, functools.partial(q_proj, 3)))
            fillers.extend(k_proj_units(3))
            fillers.append(("Q4", 8192, functools.partial(q_proj, 4)))
            fillers.extend(k_proj_units(4))
            fillers.extend(v_pass_units(2))                       # heads 8-11
            fillers.append(("Q5", 8192, functools.partial(q_proj, 5)))
            fillers.extend(k_proj_units(5))
            fillers.append(("Q6", 8192, functools.partial(q_proj, 6)))
            fillers.extend(k_proj_units(6))
            fillers.extend(v_pass_units(3))                       # heads 12-15
            fillers.append(("Q7", 8192, functools.partial(q_proj, 7)))
            fillers.extend(k_proj_units(7))

            filler_debt = [0]

            def drain_fillers(budget):
                filler_debt[0] += budget
                while fillers and filler_debt[0] >= fillers[0][1]:
                    _, cost, fn = fillers.popleft()
                    filler_debt[0] -= cost
                    fn()

            def force_prefix(pred):
                # emit fillers from the front until pred() true
                while not pred():
                    assert fillers, "deadline unsatisfiable"
                    _, cost, fn = fillers.popleft()
                    fn()

            # ---------- startup ----------
            # PE warmup: the cost model taxes matmuls dispatched within 3us
            # of a PE idle->busy edge; burn that window on throwaway matmuls
            # while the startup DMAs stream in.
            for d in range(26):
                dps = avp.tile([128, 512], F32, tag="av", name=f"warm{d}")
                nc.tensor.matmul(dps[:], warm_sb[:, 0:128], warm_sb[:],
                                 start=True, stop=True)
            nc.sync.dma_start(XQA[:].rearrange("p (i t) -> p i t", t=LQ),
                              xq_d.ap().rearrange("i p t -> p i t"))
            q_proj(0)
            xkav = XKA[:].rearrange("p (i t) -> p i t", t=L)
            for c in range(4):
                nc.sync.dma_start(
                    xkav[:, :, c * 512:(c + 1) * 512],
                    xk_d.ap()[:, :, c * 512:(c + 1) * 512].rearrange(
                        "i p t -> p i t"))
            for _, _, fn in k_proj_units(0):
                fn()

            for g in range(8):
                nc.sync.dma_start(
                    MTA[:, g * 2 * LQ:(g + 1) * 2 * LQ].rearrange(
                        "p (j t) -> p j t", t=LQ),
                    mt_d.ap()[2 * g:2 * g + 2].rearrange("j p t -> p j t"))

            # ---------- attention ----------
            etiles = {}
            cn_tiles = {}

            def scores_unit(h, j):
                m, p2 = h // 2, h % 2
                base = 32 * p2
                s = sp.tile([128, LQ], F32, tag="s", name=f"s{h}_{j}")
                k8v = K8[m][base:base + 32, :].rearrange(
                    "p (two l) -> p two l", two=2)
                q8v = Q8[m][base:base + 32, :].rearrange(
                    "p (two l) -> p two l", two=2)
                for half in range(2):
                    hs = slice(half * 512, (half + 1) * 512)
                    nc.tensor.matmul(
                        s[:, hs], k8v[:, :, j * 128:(j + 1) * 128],
                        q8v[:, :, hs], start=True, stop=True,
                        perf_mode=DR, tile_position=(base, 0))
                e = ep.tile([128, LQ], BF, tag="e", name=f"e{h}_{j}")
                nc.scalar.activation(e[:], s[:], AF.Exp, scale=0.125)
                # mask multiply: mostly DVE, some on Pool for balance
                eng = nc.gpsimd if (j % 4 == 3) else nc.vector
                eng.tensor_mul(e[:], e[:], MTA[:, j * LQ:(j + 1) * LQ])
                etiles[(h, j)] = e

            def av_group(h, t):
                av = avp.tile([128, 512], F32, tag="av", name=f"av{h}_{t}")
                slot = slice(h * SLOT, (h + 1) * SLOT)
                for jj in range(NJ):
                    nc.tensor.matmul(
                        av[:, 0:SLOT],
                        etiles[(h, jj)][:, t * 128:(t + 1) * 128],
                        VP[jj][:, slot],
                        start=(jj == 0), stop=(jj == NJ - 1))
                m, p2 = h // 2, h % 2
                if p2 == 0 and t == 0:
                    cn_tiles[m] = cnp.tile([128, LQ], BF, tag="cn",
                                           name=f"cn{m}")
                cn = cn_tiles[m]
                nc.gpsimd.tensor_scalar(
                    cn[:, t * 128 + 64 * p2:t * 128 + 64 * p2 + 64],
                    av[:, 0:DK], av[:, DK:DK + 1], None, op0=DIVOP)
                if p2 == 1 and m == NM - 1:
                    # last m: per-t transposes so out-proj unit t starts early
                    nc.scalar.dma_start(CT[m][:, t * 128:(t + 1) * 128],
                                        cn[:, t * 128:(t + 1) * 128],
                                        transpose=True)
                elif p2 == 1 and t == 7:
                    nc.scalar.dma_start(
                        CT[m][:].rearrange("p (t q) -> p t q", q=128),
                        cn[:], transpose=True)

            wos = []
            for h in range(H):
                m = h // 2
                if m > 0:
                    force_prefix(lambda: m in K8 and m in Q8)
                # two scores ahead so ScalarE has work during the AV bursts
                budget = 1900 if h < 6 else 1300
                scores_unit(h, 0)
                drain_fillers(budget)
                scores_unit(h, 1)
                drain_fillers(budget)
                if h > 0:
                    force_prefix(lambda: ((h - 1) // 4) in vdone)
                    for t in range(4):
                        av_group(h - 1, t)
                scores_unit(h, 2)
                drain_fillers(budget)
                scores_unit(h, 3)
                drain_fillers(budget)
                if h > 0:
                    for t in range(4, 8):
                        av_group(h - 1, t)
                if h == 14:
                    # reuse XKA (K-proj done) as the wo moving tiles
                    force_prefix(lambda: not fillers)
                    nc.scalar.dma_start(
                        XKA[:, 0:NI * D].rearrange("p (i d) -> p i d", d=D),
                        wo_d.ap().rearrange("i p d -> p i d"))
                    for i in range(NI):
                        wos.append(XKA[:, i * D:(i + 1) * D])
                for j in range(4, NJ):
                    scores_unit(h, j)
                    drain_fillers(budget)

            force_prefix(lambda: not fillers)
            for t in range(8):
                av_group(15, t)

            # ---------- output projection ----------
            for t in range(NM):
                po = sp.tile([128, D], F32, tag="s", name=f"po{t}")
                for half in range(2):
                    hs = slice(half * 512, (half + 1) * 512)
                    for c in range(NI):
                        nc.tensor.matmul(
                            po[:, hs], CT[c][:, t * 128:(t + 1) * 128],
                            wos[c][:, hs], start=(c == 0), stop=False)
                    nc.tensor.matmul(po[:, hs], ones_row[:], bo_sb[:, hs],
                                     start=False, stop=True)
                f = f8p.tile([128, D], BF, tag="f", name=f"f{t}")
                nc.scalar.activation(f[:], po[:], AF.Copy)
                nc.scalar.dma_start(out_d.ap()[t], f[:])

    nc.compile()
    nc._keep_tile_frees = keep
    return nc


@functools.lru_cache(maxsize=1)
def _built():
    return _build()


def _prep_core(c, q, k, v, mask01T, wqt, wkt, wvt, wot, bqt, bkt, bor):
    b, qh = c // 2, c % 2
    qs = slice(qh * LQ, (qh + 1) * LQ)
    xq = np.ascontiguousarray(q[b, qs, :].T).astype(BF16NP).reshape(NI, 128, LQ)
    xk = np.ascontiguousarray(k[b].T).astype(BF16NP).reshape(NI, 128, L)
    xv = np.ascontiguousarray(v[b].T).astype(BF16NP).reshape(NI, 128, L)
    maskt = np.ascontiguousarray(mask01T[:, qs]).reshape(NJ, 128, LQ)
    return {
        "xq": xq, "xk": xk, "xv": xv,
        "wq": wqt, "wk": wkt, "wv": wvt, "wo": wot,
        "maskt": maskt, "bqt": bqt, "bkt": bkt, "bor": bor,
    }


def kernel(q, k, v, attn_mask, Wq, bq, Wk, bk, Wv, bv, Wo, bo):
    from concourse import bass_utils

    nc = _built()

    q = np.asarray(q, np.float32)
    k = np.asarray(k, np.float32)
    v = np.asarray(v, np.float32)
    wqt = np.ascontiguousarray(np.asarray(Wq, np.float32).T).astype(BF16NP).reshape(NI, 128, D)
    wkt = np.ascontiguousarray(np.asarray(Wk, np.float32).T).astype(BF16NP).reshape(NI, 128, D)
    wvt = np.ascontiguousarray(np.asarray(Wv, np.float32).T).astype(BF16NP).reshape(NI, 128, D)
    wot = np.ascontiguousarray(np.asarray(Wo, np.float32).T).astype(BF16NP).reshape(NI, 128, D)
    mask01T = np.ascontiguousarray((np.asarray(attn_mask)[0, 0] != 0).T.astype(BF16NP))
    bqt = np.ascontiguousarray(np.asarray(bq, np.float32).reshape(NM, 128).T)
    bkt = np.ascontiguousarray(np.asarray(bk, np.float32).reshape(NM, 128).T)
    bo_eff = np.asarray(bo, np.float32) + np.asarray(Wo, np.float32) @ np.asarray(bv, np.float32)
    bor = bo_eff.astype(BF16NP).reshape(1, D)

    in_maps = [
        _prep_core(c, q, k, v, mask01T, wqt, wkt, wvt, wot, bqt, bkt, bor)
        for c in range(NCORES)
    ]
    res = bass_utils.run_bass_kernel_spmd(nc, in_maps, core_ids=list(range(NCORES)))

    out = np.empty((B, L, D), np.float32)
    for c in range(NCORES):
        b, qh = c // 2, c % 2
        out[b, qh * LQ:(qh + 1) * LQ, :] = (
            res.results[c]["out"].astype(np.float32).reshape(LQ, D))
    return out
